# revision 19
# baseline (speedup 1.0000x reference)
"""NeuralMemory fast-weight recurrence on 8 Trainium2 NeuronCores.

Sharding: 8-way tensor-parallel over the memory dim M=2048 (m_s=256/core).
Per chunk: MLP forward, analytic MSE backward, gated fast-weight update,
re-forward. Cross-core: one bf16 AllReduce per chunk carrying
(c*pred_partial - x/8); the sum across 8 cores is dpred = pred - x directly.
The final `out` is returned as per-core bf16 partial sums that the host adds.

Pipeline (iteration j): backward for chunk j (AR_j result) -> weight updates
-> forward of chunk j+1 under new params -> AR_{j+1} issue -> shadow work
during the AR flight (re-forward out_j, deferred q1n update, x_{j+2}
prefetch + PE transposes, gates for chunk j+2, scalar chain for iter j+1).

Numerics: bf16 matmul operands, fp32 PSUM. Weights kept in "Q-space"
(divided by the running forget product c_j); x is pre-scaled by 1/8 on host.
Layer-1 runs m-major (hT directly) with ACT per-partition bias; silu and
silu' are derived from sigmoid so every ACT op stays in one table set.
"""
import numpy as np
import concourse.bacc as bacc
import concourse.mybir as mybir
import concourse.tile as tile
from concourse.bass_utils import run_bass_kernel_spmd

BF = mybir.dt.bfloat16
F32 = mybir.dt.float32
AF = mybir.ActivationFunctionType
ALU = mybir.AluOpType

NCORES = 8
B, L, D, M = 2, 2048, 2048, 2048
C = 128                 # reference CHUNK
NCH = L // C            # 16 chunks
T = B * C               # 256 tokens per chunk
MS = M // NCORES        # 256 per-core memory slice
KD = D // 128           # 16 tiles over D
KT = T // 128           # 2 tiles over tokens
KM = MS // 128          # 2 tiles over m_s
NN = D // 512           # 4 N-chunks of 512 over D
LR_MEMORY = 0.01
K1 = float(-LR_MEMORY * 2.0 / (T * D) / T)   # grad scale * -eff_lr/lsum


def build(no_ar=False):
    nc = bacc.Bacc("TRN2", target_bir_lowering=False, num_devices=NCORES)
    x8 = nc.dram_tensor("x8", [B, L, D], BF, kind="ExternalInput")    # x/8, bf16
    w0t_in = nc.dram_tensor("w0t", [D, MS], F32, kind="ExternalInput")
    w1t_in = nc.dram_tensor("w1t", [MS, D], F32, kind="ExternalInput")
    w1n_in = nc.dram_tensor("w1n", [D, MS], F32, kind="ExternalInput")
    lrfg_in = nc.dram_tensor("lrfg", [D, 2], F32, kind="ExternalInput")
    lrb_in = nc.dram_tensor("lrb", [1, 1], F32, kind="ExternalInput")
    fgb_in = nc.dram_tensor("fgb", [1, 1], F32, kind="ExternalInput")
    b0c_in = nc.dram_tensor("b0c", [MS, 1], F32, kind="ExternalInput")
    b1d8_in = nc.dram_tensor("b1d8", [1, D], F32, kind="ExternalInput")
    ident_in = nc.dram_tensor("ident", [128, 128], F32, kind="ExternalInput")
    outp = nc.dram_tensor("outp", [B, L, D], BF, kind="ExternalOutput")

    with tile.TileContext(nc) as tc:
        with (
            tc.tile_pool(name="wp", bufs=1) as wp,          # persistent weights/consts
            tc.tile_pool(name="xp", bufs=3) as xp,          # x streams (3 chunks live)
            tc.tile_pool(name="hp", bufs=2) as hpool,       # per-chunk h state
            tc.tile_pool(name="tp", bufs=2) as tp,          # within-iteration temps
            tc.tile_pool(name="sp", bufs=2) as spool,       # tiny scalar tiles
            tc.tile_pool(name="psA", bufs=2, space="PSUM") as psA,   # [128,512] mm2/gW1t
            tc.tile_pool(name="psB", bufs=2, space="PSUM") as psB,   # [128,256] mm1T/dh/gW0
            tc.tile_pool(name="psD", bufs=1, space="PSUM") as psD,   # small rows
            tc.tile_pool(name="psO", bufs=1, space="PSUM") as psO,   # shadow out-fwd
            tc.tile_pool(name="psT", bufs=2, space="PSUM") as psT,   # transpose groups
            tc.tile_pool(name="dr", bufs=2, space="DRAM") as dr,
        ):
            # ---------------- persistent weights (bf16, Q-space) ----------------
            q0t = wp.tile([128, KD * MS], BF, name="q0t")      # W0^T: d-tile i at cols i*MS
            q1t = wp.tile([128, KM * D], BF, name="q1t")       # W1^T: m-tile k at cols k*D
            q1n = wp.tile([128, KD * MS], BF, name="q1n")      # W1:   d-tile i at cols i*MS
            qb0c = wp.tile([128, KM], F32, name="qb0c")        # b0 column, k-tile per col
            bk1 = wp.tile([128, D], BF, name="bk1")            # row0 = qb1/8
            lrfg = wp.tile([128, KD * 2], BF, name="lrfg")     # d-tile i at cols 2i..2i+1
            ones_row = wp.tile([128, 128], BF, name="ones_row")  # row0 = 1, rest 0
            ones_col = wp.tile([128, 1], BF, name="ones_col")    # all ones
            ones_t = wp.tile([128, 256], BF, name="ones_t")      # all ones
            ident = wp.tile([128, 128], BF, name="ident")        # identity (PE transpose)
            ones_1r = wp.tile([1, 128], F32, name="ones_1r")   # f32 ones row
            lrb_sb = wp.tile([1, 1], F32, name="lrb_sb")
            fgb_sb = wp.tile([1, 1], F32, name="fgb_sb")

            w0t3 = w0t_in.rearrange("(i p) m -> i p m", p=128)
            w1t3 = w1t_in.rearrange("(k p) d -> k p d", p=128)
            w1n3 = w1n_in.rearrange("(i p) m -> i p m", p=128)
            lrfg3 = lrfg_in.rearrange("(i p) g -> i p g", p=128)
            b0c3 = b0c_in.rearrange("(k p) o -> k p o", p=128)
            for i in range(KD):
                nc.gpsimd.dma_start(q0t[:, i * MS:(i + 1) * MS], w0t3[i])
                nc.gpsimd.dma_start(q1n[:, i * MS:(i + 1) * MS], w1n3[i])
                nc.gpsimd.dma_start(lrfg[:, 2 * i:2 * i + 2], lrfg3[i])
            for k in range(KM):
                nc.gpsimd.dma_start(q1t[:, k * D:(k + 1) * D], w1t3[k])
                nc.sync.dma_start(qb0c[:, k:k + 1], b0c3[k])
            nc.gpsimd.memset(bk1[:], 0.0)
            nc.gpsimd.dma_start(bk1[0:1, :], b1d8_in[:])
            nc.gpsimd.memset(ones_row[:], 0.0)
            nc.vector.memset(ones_row[0:1, :], 1.0)
            nc.vector.memset(ones_col[:], 1.0)
            nc.vector.memset(ones_t[:], 1.0)
            nc.vector.memset(ones_1r[:], 1.0)
            nc.gpsimd.dma_start(ident[:], ident_in[:])
            nc.sync.dma_start(lrb_sb[:], lrb_in[:])
            nc.sync.dma_start(fgb_sb[:], fgb_in[:])

            # ---------------- helpers ----------------
            def load_x(j):
                """DMA x/8 chunk j into a bf16 tile [128, KT*D] (tokens-major)."""
                xb = xp.tile([128, KT * D], BF, name=f"xb8_{j}", tag="xb8")
                for t in range(KT):
                    eng = nc.sync if t == 0 else nc.scalar
                    eng.dma_start(xb[:, t * D:(t + 1) * D],
                                  x8[t, j * C:(j + 1) * C, :])
                return xb

            def transpose_x(xb, j):
                """PE-transpose tokens-major -> d-major, grouped evacuation."""
                xT = xp.tile([128, KD * T], BF, name=f"xT8_{j}", tag="xT8")
                for i2 in range(KD // 2):           # pairs of d-tiles -> 1 bank
                    pt = psT.tile([128, 512], BF, name=f"ptx{j}_{i2}", tag="psT")
                    for q in range(4):
                        i = 2 * i2 + q // 2
                        t = q % 2
                        nc.tensor.transpose(pt[:, q * 128:(q + 1) * 128],
                                            xb[:, t * D + i * 128:t * D + (i + 1) * 128],
                                            ident[:])
                    if i2 % 2 == 0:
                        nc.vector.tensor_copy(xT[:, i2 * 512:(i2 + 1) * 512], pt[:])
                    else:
                        nc.scalar.copy(xT[:, i2 * 512:(i2 + 1) * 512], pt[:])
                return xT

            def gates_and_scalars(xT, j, c11):
                """Gate matmuls + sigmoids for chunk j, then the full scalar chain
                for iteration j. Returns dict of scalars + new c11 (= cn_j)."""
                g2ab = psD.tile([2, T], F32, name=f"g2ab_{j}", tag="psD")
                for i in range(KD):
                    nc.tensor.matmul(g2ab[:], lrfg[:, 2 * i:2 * i + 2],
                                     xT[:, i * T:(i + 1) * T],
                                     start=(i == 0), stop=(i == KD - 1))
                sigl = spool.tile([1, T], F32, name=f"sigl{j}", tag="sigl")
                lsum = spool.tile([1, 1], F32, name=f"lsum{j}", tag="lsum")
                nc.scalar.activation(sigl[:], g2ab[0:1, :], AF.Sigmoid,
                                     bias=lrb_sb[0:1, 0:1], scale=8.0,
                                     accum_out=lsum[:])
                fparts = spool.tile([1, 2], F32, name=f"fparts{j}", tag="fparts")
                for b in range(B):
                    r = spool.tile([1, 1], F32, name=f"zfr{j}_{b}", tag=f"zfr{b}")
                    nc.vector.tensor_reduce(r[:], g2ab[1:2, b * C:(b + 1) * C],
                                            mybir.AxisListType.X, ALU.add)
                    nc.scalar.activation(fparts[:, b:b + 1], r[:], AF.Sigmoid,
                                         bias=fgb_sb[0:1, 0:1], scale=8.0 / C)
                f11 = spool.tile([1, 1], F32, name=f"f11_{j}", tag="f11")
                nc.vector.tensor_reduce(f11[:], fparts[:], mybir.AxisListType.X, ALU.add)
                nc.vector.tensor_scalar_mul(f11[:], f11[:], 0.5)

                # scalar row: 0=negs 1=negs0 2=negs08 3=cn 4=c8n 5=-rcn
                scrow = spool.tile([1, 6], F32, name=f"scrow{j}", tag="scrow")
                nc.vector.tensor_tensor(scrow[0:1, 3:4], c11[:], f11[:], ALU.mult)
                rcn = spool.tile([1, 1], F32, name=f"rcn{j}", tag="rcn")
                nc.vector.reciprocal(rcn[:], scrow[0:1, 3:4])
                nc.vector.tensor_tensor(scrow[0:1, 0:1], lsum[:], rcn[:], ALU.mult)
                nc.vector.tensor_scalar_mul(scrow[0:1, 0:1], scrow[0:1, 0:1], K1)
                negs8 = spool.tile([1, 1], F32, name=f"negs8_{j}", tag="negs8")
                nc.vector.tensor_scalar_mul(negs8[:], scrow[0:1, 0:1], 1.0 / 8.0)
                nc.vector.tensor_tensor(scrow[0:1, 1:2], scrow[0:1, 0:1], c11[:], ALU.mult)
                nc.vector.tensor_scalar_mul(scrow[0:1, 2:3], scrow[0:1, 1:2], 8.0)
                nc.vector.tensor_scalar_mul(scrow[0:1, 4:5], scrow[0:1, 3:4], 8.0)
                nc.vector.tensor_scalar_mul(scrow[0:1, 5:6], rcn[:], -1.0)
                # broadcast all six down the partitions with one PE matmul
                psbc = psD.tile([128, 6], F32, name=f"psbc{j}", tag="psD")
                nc.tensor.matmul(psbc[:], ones_1r[0:1, :], scrow[0:1, :],
                                 start=True, stop=True)
                bcs = spool.tile([128, 6], F32, name=f"bcs{j}", tag="bcs")
                nc.vector.tensor_copy(bcs[:], psbc[:])
                identm = tp.tile([128, 128], BF, name=f"idm{j}", tag="identm")
                nc.scalar.activation(identm[:], ident[:], AF.Copy,
                                     scale=bcs[:, 5:6])
                return dict(cn11=scrow[0:1, 3:4], negs8=negs8, bcs=bcs,
                            identm=identm)

            def fwd1(xT, sc, cqb, want_hp, j, pfx, pspool=None, pstag="psB"):
                """Layer-1 m-major: hT [m,(k)*T], optional hpT. ACT per-partition
                bias; silu & silu' derived from sigmoid."""
                if pspool is None:
                    pspool = psB
                hT = hpool.tile([128, KM * T], BF, name=f"hT{pfx}_{j}", tag=f"hT{pfx}")
                hpT = None
                if want_hp:
                    hpT = hpool.tile([128, KM * T], BF, name=f"hpT_{j}", tag="hpT")
                for k in range(KM):
                    pt = pspool.tile([128, T], F32, name=f"ps1{pfx}_{j}_{k}", tag=pstag)
                    for i in range(KD):
                        nc.tensor.matmul(pt[:],
                                         q0t[:, i * MS + k * 128:i * MS + (k + 1) * 128],
                                         xT[:, i * T:(i + 1) * T],
                                         start=(i == 0), stop=(i == KD - 1))
                    sl = slice(k * T, (k + 1) * T)
                    sT = tp.tile([128, T], BF, name=f"sT{pfx}_{j}_{k}", tag=f"sT{k}")
                    nc.scalar.activation(sT[:], pt[:], AF.Sigmoid,
                                         bias=cqb[:, k:k + 1], scale=sc["bcs"][:, 4:5])
                    pre = tp.tile([128, T], BF, name=f"pre{pfx}_{j}_{k}", tag=f"pre{k}")
                    nc.scalar.activation(pre[:], pt[:], AF.Identity,
                                         bias=cqb[:, k:k + 1], scale=sc["bcs"][:, 4:5])
                    nc.vector.tensor_tensor(hT[:, sl], pre[:], sT[:], ALU.mult)
                    if want_hp:
                        m_ = tp.tile([128, T], BF, name=f"m{pfx}_{j}_{k}", tag=f"mm{k}")
                        nc.vector.tensor_tensor(m_[:], hT[:, sl], sT[:], ALU.mult)
                        v_ = tp.tile([128, T], BF, name=f"v{pfx}_{j}_{k}", tag=f"vv{k}")
                        nc.vector.tensor_tensor(v_[:], sT[:], m_[:], ALU.subtract)
                        nc.vector.tensor_tensor(hpT[:, sl], v_[:], hT[:, sl], ALU.add)
                return hT, hpT

            def fwd2_pred(hT, sc, xb_next, j):
                """Layer-2 tokens-major for pred: evac (cn*psum - x/8) -> arin DMA,
                issue AllReduce. Returns arout handle."""
                psb = tp.tile([128, KT * D], BF, name=f"parin{j}", tag="parin")
                arin = dr.tile([T, D], BF, name=f"arin{j}", tag="arin")
                for t in range(KT):
                    for n in range(NN):
                        pt = psA.tile([128, 512], F32, name=f"psp{j}_{t}_{n}", tag="psA")
                        for k in range(KM):
                            nc.tensor.matmul(pt[:], hT[:, k * T + t * 128:k * T + (t + 1) * 128],
                                             q1t[:, k * D + n * 512:k * D + (n + 1) * 512],
                                             start=(k == 0), stop=False)
                        nc.tensor.matmul(pt[:], ones_row[:], bk1[:, n * 512:(n + 1) * 512],
                                         start=False, stop=False)
                        sl = slice(t * D + n * 512, t * D + (n + 1) * 512)
                        nc.tensor.matmul(pt[:], sc["identm"][:], xb_next[:, sl],
                                         start=False, stop=True)
                        if n % 2 == 0:
                            nc.vector.tensor_scalar_mul(psb[:, sl], pt[:],
                                                        sc["bcs"][:, 3:4])
                        else:
                            nc.scalar.mul(psb[:, sl], pt[:], sc["bcs"][:, 3:4])
                    eng = nc.gpsimd if t == 0 else nc.sync
                    eng.dma_start(arin[t * 128:(t + 1) * 128, :],
                                  psb[:, t * D:(t + 1) * D])
                if no_ar:
                    return arin
                arout = dr.tile([T, D], BF, name=f"arout{j}", tag="arout",
                                addr_space="Shared")
                nc.gpsimd.collective_compute(
                    "AllReduce", ALU.add, replica_groups=[list(range(NCORES))],
                    ins=[arin.opt()], outs=[arout.opt()])
                return arout

            def fwd2_out(hT, sc, j):
                """Layer-2 tokens-major for out_j: evac cn*psum -> bf16 -> outp."""
                osb = tp.tile([128, KT * D], BF, name=f"osb{j}", tag="osb")
                for t in range(KT):
                    for n in range(NN):
                        pt = psO.tile([128, 512], F32, name=f"pso{j}_{t}_{n}", tag="psO")
                        for k in range(KM):
                            nc.tensor.matmul(pt[:], hT[:, k * T + t * 128:k * T + (t + 1) * 128],
                                             q1t[:, k * D + n * 512:k * D + (n + 1) * 512],
                                             start=(k == 0), stop=False)
                        nc.tensor.matmul(pt[:], ones_row[:], bk1[:, n * 512:(n + 1) * 512],
                                         start=False, stop=True)
                        sl = slice(t * D + n * 512, t * D + (n + 1) * 512)
                        nc.scalar.activation(osb[:, sl], pt[:], AF.Copy,
                                             scale=sc["bcs"][:, 3:4])
                    nc.sync.dma_start(outp[t, j * C:(j + 1) * C, :],
                                      osb[:, t * D:(t + 1) * D])

            def transpose_h(hT, j):
                """hT [m,(k)T] -> h tokens-major [t, KT*MS] (one bank, one evac)."""
                h1 = hpool.tile([128, KT * MS], BF, name=f"h1_{j}", tag="h1")
                pt = psT.tile([128, 512], BF, name=f"pth{j}", tag="psT")
                for q in range(4):      # (t,k): dst col t*MS+k*128 contiguous
                    t, k = q // 2, q % 2
                    nc.tensor.transpose(pt[:, t * MS + k * 128:t * MS + k * 128 + 128],
                                        hT[:, k * T + t * 128:k * T + (t + 1) * 128],
                                        ident[:])
                nc.vector.tensor_copy(h1[:], pt[:])
                return h1

            # ---------------- prologue ----------------
            c11 = spool.tile([1, 1], F32, name="c11", tag="c11")
            nc.vector.memset(c11[:], 1.0)
            xb_c = load_x(0)
            xT_c = transpose_x(xb_c, 0)
            sc_c = gates_and_scalars(xT_c, 0, c11)      # scalars for iteration 0
            c11 = sc_c["cn11"]
            # chunk 0 forward under P_0 (c=1): scale 8, bias qb0c
            bcs0 = spool.tile([128, 6], F32, name="bcs0")
            nc.vector.memset(bcs0[:], 0.0)
            nc.vector.memset(bcs0[:, 3:4], 1.0)
            nc.vector.memset(bcs0[:, 4:5], 8.0)
            nc.vector.memset(bcs0[:, 5:6], -1.0)
            identm0 = tp.tile([128, 128], BF, name="idm_p", tag="identm")
            nc.scalar.activation(identm0[:], ident[:], AF.Copy, scale=-1.0)
            sc0 = dict(sc_c)
            sc0["bcs"] = bcs0
            sc0["identm"] = identm0

            xb_n = load_x(1)
            xT_n = transpose_x(xb_n, 1)
            sc_n = gates_and_scalars(xT_n, 1, c11)      # scalars for iteration 1
            c11 = sc_n["cn11"]
            cqb = spool.tile([128, KM], F32, name="cqb_p", tag="cqb")
            nc.vector.tensor_copy(cqb[:], qb0c[:])
            hT_c, hpT_c = fwd1(xT_c, sc0, cqb, True, 0, "1")
            h1_c = transpose_h(hT_c, 0)
            ar_cur = fwd2_pred(hT_c, sc0, xb_c, 0)

            xb_f = load_x(2)
            xT_f = transpose_x(xb_f, 2)

            # ---------------- main loop ----------------
            for j in range(NCH):
                last = (j == NCH - 1)
                sc = sc_c   # scalars for iteration j (gates of chunk j)

                # AllReduce result -> SBUF (dpred, tokens-major)
                dpred = tp.tile([128, KT * D], BF, name=f"dp{j}", tag="dp")
                for t in range(KT):
                    eng = nc.sync if t == 0 else nc.scalar
                    eng.dma_start(dpred[:, t * D:(t + 1) * D],
                                  ar_cur[t * 128:(t + 1) * 128, :])

                # dpredT (d-major) via grouped PE transposes
                dpT = tp.tile([128, KD * T], BF, name=f"dpT{j}", tag="dpT")
                for i2 in range(KD // 2):
                    pt = psT.tile([128, 512], BF, name=f"ptd{j}_{i2}", tag="psT")
                    for q in range(4):
                        i = 2 * i2 + q // 2
                        t = q % 2
                        nc.tensor.transpose(pt[:, q * 128:(q + 1) * 128],
                                            dpred[:, t * D + i * 128:t * D + (i + 1) * 128],
                                            ident[:])
                    with tc.high_priority():
                        if i2 % 2 == 0:
                            nc.vector.tensor_copy(dpT[:, i2 * 512:(i2 + 1) * 512], pt[:])
                        else:
                            nc.scalar.copy(dpT[:, i2 * 512:(i2 + 1) * 512], pt[:])

                # dhT (m-major) = q1n^T-contraction; dhpT = dhT * hpT
                dhpT = tp.tile([128, KM * T], BF, name=f"dhpT{j}", tag="dhpT")
                for k in range(KM):
                    pt = psB.tile([128, T], F32, name=f"psdh{j}_{k}", tag="psB")
                    for i in range(KD):
                        nc.tensor.matmul(pt[:],
                                         q1n[:, i * MS + k * 128:i * MS + (k + 1) * 128],
                                         dpT[:, i * T:(i + 1) * T],
                                         start=(i == 0), stop=(i == KD - 1))
                    sl = slice(k * T, (k + 1) * T)
                    with tc.high_priority():
                        nc.vector.tensor_tensor(dhpT[:, sl], pt[:], hpT_c[:, sl], ALU.mult)

                # gb0 column via DVE free-axis reduce of dhpT; update qb0c; cqb0
                gb0c = spool.tile([128, KM], F32, name=f"gb0c{j}", tag="gb0c")
                cqb = spool.tile([128, KM], F32, name=f"cqb{j}", tag="cqb")
                with tc.high_priority():
                    for k in range(KM):
                        nc.vector.tensor_reduce(gb0c[:, k:k + 1], dhpT[:, k * T:(k + 1) * T],
                                                mybir.AxisListType.X, ALU.add)
                    nc.vector.scalar_tensor_tensor(qb0c[:], gb0c[:], sc["bcs"][:, 1:2],
                                                   qb0c[:], ALU.mult, ALU.add)
                    nc.vector.scalar_tensor_tensor(cqb[:], qb0c[:], sc["bcs"][:, 3:4],
                                                   ones_t[:, 0:KM], ALU.mult, ALU.mult)

                # dhp tokens-major (one bank, one evac)
                dhp = tp.tile([128, KT * MS], BF, name=f"dhp{j}", tag="dhp")
                ptd = psT.tile([128, 512], BF, name=f"ptdh{j}", tag="psT")
                for q in range(4):
                    t, k = q // 2, q % 2
                    nc.tensor.transpose(ptd[:, t * MS + k * 128:t * MS + k * 128 + 128],
                                        dhpT[:, k * T + t * 128:k * T + (t + 1) * 128],
                                        ident[:])
                with tc.high_priority():
                    nc.vector.tensor_copy(dhp[:], ptd[:])

                # gW0 (q0t layout) fused updates (x/8 lhsT -> scale *8)
                for i in range(KD):
                    pt = psB.tile([128, MS], F32, name=f"psg0_{j}_{i}", tag="psB")
                    for t in range(KT):
                        nc.tensor.matmul(pt[:], xb_c[:, t * D + i * 128:t * D + (i + 1) * 128],
                                         dhp[:, t * MS:(t + 1) * MS],
                                         start=(t == 0), stop=(t == KT - 1))
                    sl = slice(i * MS, (i + 1) * MS)
                    if i % 2 == 0:
                        nc.vector.scalar_tensor_tensor(q0t[:, sl], pt[:],
                                                       sc["bcs"][:, 2:3],
                                                       q0t[:, sl], ALU.mult, ALU.add)
                    else:
                        gsc = tp.tile([128, MS], BF, name=f"g0s{j}_{i}", tag="g0s")
                        nc.scalar.mul(gsc[:], pt[:], sc["bcs"][:, 2:3])
                        nc.vector.tensor_tensor(q0t[:, sl], gsc[:], q0t[:, sl],
                                                ALU.add)

                # ---- forward chunk j+1 under P_{j+1} (layer 1) ----
                if not last:
                    hT_n, hpT_n = fwd1(xT_n, sc, cqb, True, j + 1, "1")

                # gW1t (q1t layout) + gb1 from dpred/h1 -> fused updates
                # (only mm2 needs these; scheduled after fwd1 so the DVE queue
                # drains the layer-1 critical chain first)
                for k in range(KM):
                    for n in range(NN):
                        pt = psA.tile([128, 512], F32, name=f"psg1_{j}_{k}_{n}", tag="psA")
                        for t in range(KT):
                            nc.tensor.matmul(pt[:],
                                             h1_c[:, t * MS + k * 128:t * MS + (k + 1) * 128],
                                             dpred[:, t * D + n * 512:t * D + (n + 1) * 512],
                                             start=(t == 0), stop=(t == KT - 1))
                        sl = slice(k * D + n * 512, k * D + (n + 1) * 512)
                        if n % 2 == 0:
                            nc.vector.scalar_tensor_tensor(q1t[:, sl], pt[:],
                                                           sc["bcs"][:, 0:1],
                                                           q1t[:, sl], ALU.mult, ALU.add)
                        else:
                            g1s = tp.tile([128, 512], BF, name=f"g1s{j}_{k}_{n}", tag="g1s")
                            nc.scalar.mul(g1s[:], pt[:], sc["bcs"][:, 0:1])
                            nc.vector.tensor_tensor(q1t[:, sl], g1s[:], q1t[:, sl],
                                                    ALU.add)
                for n in range(NN):
                    gb1p = psD.tile([1, 512], F32, name=f"gb1_{j}_{n}", tag="psD")
                    for t in range(KT):
                        nc.tensor.matmul(gb1p[:], ones_col[:],
                                         dpred[:, t * D + n * 512:t * D + (n + 1) * 512],
                                         start=(t == 0), stop=(t == KT - 1))
                    nc.vector.scalar_tensor_tensor(bk1[0:1, n * 512:(n + 1) * 512],
                                                   gb1p[:], sc["negs8"][0:1, 0:1],
                                                   bk1[0:1, n * 512:(n + 1) * 512],
                                                   ALU.mult, ALU.add)

                # ---- layer 2 for pred_{j+1}; issue AR ----
                if not last:
                    ar_cur = fwd2_pred(hT_n, sc, xb_n, j + 1)

                # ---- shadow work (during AR flight) ----
                # re-forward chunk j -> out_j
                hT_o, _ = fwd1(xT_c, sc, cqb, False, j, "2", pspool=psO, pstag="psO")
                fwd2_out(hT_o, sc, j)

                # refresh q1n by transposing the updated q1t (needed only by
                # the next backward; exact same bf16 values as a dual update)
                for i2 in range(KD // 2):
                    pt = psT.tile([128, 512], BF, name=f"ptq{j}_{i2}", tag="psT")
                    for q in range(4):
                        i = 2 * i2 + q // 2
                        k = q % 2
                        nc.tensor.transpose(pt[:, q * 128:(q + 1) * 128],
                                            q1t[:, k * D + i * 128:k * D + (i + 1) * 128],
                                            ident[:])
                    if i2 % 2 == 0:
                        nc.vector.tensor_copy(q1n[:, i2 * 512:(i2 + 1) * 512], pt[:])
                    else:
                        nc.scalar.copy(q1n[:, i2 * 512:(i2 + 1) * 512], pt[:])

                if not last:
                    # h1 for next backward; prefetch + gates two chunks ahead
                    h1_n = transpose_h(hT_n, j + 1)
                    if j < NCH - 2:
                        sc_f = gates_and_scalars(xT_f, j + 2, c11)
                        c11 = sc_f["cn11"]
                        if j < NCH - 3:
                            xb_2 = load_x(j + 3)
                            xT_2 = transpose_x(xb_2, j + 3)
                        else:
                            xb_2 = xT_2 = None
                    else:
                        sc_f = xb_2 = xT_2 = None
                    xb_c, xT_c = xb_n, xT_n
                    xb_n, xT_n = xb_f, xT_f
                    xb_f, xT_f = xb_2, xT_2
                    hT_c, hpT_c, h1_c = hT_n, hpT_n, h1_n
                    sc_c, sc_n = sc_n, sc_f
    nc.compile()
    return nc


_NC_CACHE = None


def _get_nc():
    global _NC_CACHE
    if _NC_CACHE is None:
        _NC_CACHE = build()
    return _NC_CACHE


def make_in_maps(x, W0, b0, W1, b1, lr_w, lr_b, fg_w, fg_b):
    import ml_dtypes
    x8 = np.ascontiguousarray(
        (np.asarray(x, np.float32) * 0.125).astype(ml_dtypes.bfloat16))
    W0 = np.asarray(W0, np.float32)
    W1 = np.asarray(W1, np.float32)
    lrfg = np.ascontiguousarray(
        np.stack([np.asarray(lr_w, np.float32)[0], np.asarray(fg_w, np.float32)[0]], axis=1))
    ident = np.eye(128, dtype=np.float32)
    in_maps = []
    for s in range(NCORES):
        sl = slice(s * MS, (s + 1) * MS)
        in_maps.append({
            "x8": x8,
            "w0t": np.ascontiguousarray(W0[sl, :].T),
            "w1t": np.ascontiguousarray(W1[:, sl].T),
            "w1n": np.ascontiguousarray(W1[:, sl]),
            "lrfg": lrfg,
            "lrb": np.asarray(lr_b, np.float32).reshape(1, 1),
            "fgb": np.asarray(fg_b, np.float32).reshape(1, 1),
            "b0c": np.ascontiguousarray(np.asarray(b0, np.float32)[sl].reshape(MS, 1)),
            "b1d8": np.ascontiguousarray((np.asarray(b1, np.float32) / 8.0).reshape(1, D)),
            "ident": ident,
        })
    return in_maps


def run(inputs, **kw):
    nc = _get_nc()
    in_maps = make_in_maps(**inputs)
    res = run_bass_kernel_spmd(nc, in_maps, core_ids=list(range(NCORES)), **kw)
    out = np.zeros((B, L, D), np.float32)
    for r in res.results:
        out += r["outp"].astype(np.float32)
    return out, res


def kernel(**inputs) -> np.ndarray:
    out, _ = run(inputs)
    return out


# revision 22
# speedup vs baseline: 1.5808x; 1.5808x over previous
"""NeuralMemory fast-weight recurrence on 8 Trainium2 NeuronCores.

Sharding: 8-way tensor-parallel over the memory dim M=2048 (m_s=256/core).
Per chunk: MLP forward, analytic MSE backward, gated fast-weight update,
re-forward. Cross-core: one bf16 AllReduce per chunk carrying
(c*pred_partial - x/8); the sum across 8 cores is dpred = pred - x directly.
The final `out` is returned as per-core bf16 partial sums that the host adds.

Pipeline (iteration j): backward for chunk j (AR_j result) -> weight updates
-> forward of chunk j+1 under new params -> AR_{j+1} issue -> shadow work
during the AR flight (re-forward out_j, deferred q1n update, x_{j+2}
prefetch + PE transposes, gates for chunk j+2, scalar chain for iter j+1).

Numerics: bf16 matmul operands, fp32 PSUM. Weights kept in "Q-space"
(divided by the running forget product c_j); x is pre-scaled by 1/8 on host.
Layer-1 runs m-major (hT directly) with ACT per-partition bias; silu and
silu' are derived from sigmoid so every ACT op stays in one table set.
"""
import numpy as np
import concourse.bacc as bacc
import concourse.mybir as mybir
import concourse.tile as tile
from concourse.bass_utils import run_bass_kernel_spmd

BF = mybir.dt.bfloat16
F32 = mybir.dt.float32
AF = mybir.ActivationFunctionType
ALU = mybir.AluOpType

NCORES = 8
B, L, D, M = 2, 2048, 2048, 2048
C = 128                 # reference CHUNK
NCH = L // C            # 16 chunks
T = B * C               # 256 tokens per chunk
MS = M // NCORES        # 256 per-core memory slice
KD = D // 128           # 16 tiles over D
KT = T // 128           # 2 tiles over tokens
KM = MS // 128          # 2 tiles over m_s
NN = D // 512           # 4 N-chunks of 512 over D
LR_MEMORY = 0.01
K1 = float(-LR_MEMORY * 2.0 / (T * D) / T)   # grad scale * -eff_lr/lsum


def build(no_ar=False, ar_fp8=False):
    ARDT = mybir.dt.float8e4 if ar_fp8 else BF
    nc = bacc.Bacc("TRN2", target_bir_lowering=False, num_devices=NCORES)
    x8 = nc.dram_tensor("x8", [B, L, D], BF, kind="ExternalInput")    # x/8, bf16
    w0t_in = nc.dram_tensor("w0t", [D, MS], F32, kind="ExternalInput")
    w1t_in = nc.dram_tensor("w1t", [MS, D], F32, kind="ExternalInput")
    w1n_in = nc.dram_tensor("w1n", [D, MS], F32, kind="ExternalInput")
    lrfg_in = nc.dram_tensor("lrfg", [D, 2], F32, kind="ExternalInput")
    lrb_in = nc.dram_tensor("lrb", [1, 1], F32, kind="ExternalInput")
    fgb_in = nc.dram_tensor("fgb", [1, 1], F32, kind="ExternalInput")
    b0c_in = nc.dram_tensor("b0c", [MS, 1], F32, kind="ExternalInput")
    b1d8_in = nc.dram_tensor("b1d8", [1, D], F32, kind="ExternalInput")
    ident_in = nc.dram_tensor("ident", [128, 128], F32, kind="ExternalInput")
    outp = nc.dram_tensor("outp", [B, L, D], BF, kind="ExternalOutput")

    with tile.TileContext(nc) as tc:
        with (
            tc.tile_pool(name="wp", bufs=1) as wp,          # persistent weights/consts
            tc.tile_pool(name="xp", bufs=3) as xp,          # x streams (3 chunks live)
            tc.tile_pool(name="hp", bufs=2) as hpool,       # per-chunk h state
            tc.tile_pool(name="tp", bufs=2) as tp,          # within-iteration temps
            tc.tile_pool(name="sp", bufs=2) as spool,       # tiny scalar tiles
            tc.tile_pool(name="psA", bufs=2, space="PSUM") as psA,   # [128,512] mm2/gW1t
            tc.tile_pool(name="psB", bufs=2, space="PSUM") as psB,   # [128,256] mm1T/dh/gW0
            tc.tile_pool(name="psD", bufs=1, space="PSUM") as psD,   # small rows
            tc.tile_pool(name="psO", bufs=1, space="PSUM") as psO,   # shadow out-fwd
            tc.tile_pool(name="psT", bufs=2, space="PSUM") as psT,   # transpose groups
            tc.tile_pool(name="dr", bufs=2, space="DRAM") as dr,
        ):
            # ---------------- persistent weights (bf16, Q-space) ----------------
            q0t = wp.tile([128, KD * MS], BF, name="q0t")      # W0^T: d-tile i at cols i*MS
            q1t = wp.tile([128, KM * D], BF, name="q1t")       # W1^T: m-tile k at cols k*D
            q1n = wp.tile([128, KD * MS], BF, name="q1n")      # W1:   d-tile i at cols i*MS
            qb0c = wp.tile([128, KM], F32, name="qb0c")        # b0 column, k-tile per col
            bk1 = wp.tile([128, D], BF, name="bk1")            # row0 = qb1/8
            lrfg = wp.tile([128, KD * 2], BF, name="lrfg")     # d-tile i at cols 2i..2i+1
            ones_row = wp.tile([128, 128], BF, name="ones_row")  # row0 = 1, rest 0
            ones_col = wp.tile([128, 1], BF, name="ones_col")    # all ones
            ones_t = wp.tile([128, 256], BF, name="ones_t")      # all ones
            ident = wp.tile([128, 128], BF, name="ident")        # identity (PE transpose)
            ones_1r = wp.tile([1, 128], F32, name="ones_1r")   # f32 ones row
            lrb_sb = wp.tile([1, 1], F32, name="lrb_sb")
            fgb_sb = wp.tile([1, 1], F32, name="fgb_sb")

            w0t3 = w0t_in.rearrange("(i p) m -> i p m", p=128)
            w1t3 = w1t_in.rearrange("(k p) d -> k p d", p=128)
            w1n3 = w1n_in.rearrange("(i p) m -> i p m", p=128)
            lrfg3 = lrfg_in.rearrange("(i p) g -> i p g", p=128)
            b0c3 = b0c_in.rearrange("(k p) o -> k p o", p=128)
            for i in range(KD):
                nc.gpsimd.dma_start(q0t[:, i * MS:(i + 1) * MS], w0t3[i])
                nc.gpsimd.dma_start(q1n[:, i * MS:(i + 1) * MS], w1n3[i])
                nc.gpsimd.dma_start(lrfg[:, 2 * i:2 * i + 2], lrfg3[i])
            for k in range(KM):
                nc.gpsimd.dma_start(q1t[:, k * D:(k + 1) * D], w1t3[k])
                nc.sync.dma_start(qb0c[:, k:k + 1], b0c3[k])
            nc.gpsimd.memset(bk1[:], 0.0)
            nc.gpsimd.dma_start(bk1[0:1, :], b1d8_in[:])
            nc.gpsimd.memset(ones_row[:], 0.0)
            nc.vector.memset(ones_row[0:1, :], 1.0)
            nc.vector.memset(ones_col[:], 1.0)
            nc.vector.memset(ones_t[:], 1.0)
            nc.vector.memset(ones_1r[:], 1.0)
            nc.gpsimd.dma_start(ident[:], ident_in[:])
            nc.sync.dma_start(lrb_sb[:], lrb_in[:])
            nc.sync.dma_start(fgb_sb[:], fgb_in[:])

            # ---------------- helpers ----------------
            def load_x(j):
                """DMA x/8 chunk j into a bf16 tile [128, KT*D] (tokens-major)."""
                xb = xp.tile([128, KT * D], BF, name=f"xb8_{j}", tag="xb8")
                for t in range(KT):
                    eng = nc.sync if t == 0 else nc.scalar
                    eng.dma_start(xb[:, t * D:(t + 1) * D],
                                  x8[t, j * C:(j + 1) * C, :])
                return xb

            def transpose_x(xb, j):
                """PE-transpose tokens-major -> d-major, grouped evacuation."""
                xT = xp.tile([128, KD * T], BF, name=f"xT8_{j}", tag="xT8")
                for i2 in range(KD // 2):           # pairs of d-tiles -> 1 bank
                    pt = psT.tile([128, 512], BF, name=f"ptx{j}_{i2}", tag="psT")
                    for q in range(4):
                        i = 2 * i2 + q // 2
                        t = q % 2
                        nc.tensor.transpose(pt[:, q * 128:(q + 1) * 128],
                                            xb[:, t * D + i * 128:t * D + (i + 1) * 128],
                                            ident[:])
                    if i2 % 2 == 0:
                        nc.vector.tensor_copy(xT[:, i2 * 512:(i2 + 1) * 512], pt[:])
                    else:
                        nc.scalar.copy(xT[:, i2 * 512:(i2 + 1) * 512], pt[:])
                return xT

            def gates_and_scalars(xT, j, c11):
                """Gate matmuls + sigmoids for chunk j, then the full scalar chain
                for iteration j. Returns dict of scalars + new c11 (= cn_j)."""
                g2a = psD.tile([1, T], F32, name=f"g2a_{j}", tag="psD")
                g2b = psD.tile([1, T], F32, name=f"g2b_{j}", tag="psD")
                for i in range(KD):
                    nc.tensor.matmul(g2a[:], lrfg[:, 2 * i:2 * i + 1],
                                     xT[:, i * T:(i + 1) * T],
                                     start=(i == 0), stop=(i == KD - 1))
                for i in range(KD):
                    nc.tensor.matmul(g2b[:], lrfg[:, 2 * i + 1:2 * i + 2],
                                     xT[:, i * T:(i + 1) * T],
                                     start=(i == 0), stop=(i == KD - 1))
                sigl = spool.tile([1, T], F32, name=f"sigl{j}", tag="sigl")
                lsum = spool.tile([1, 1], F32, name=f"lsum{j}", tag="lsum")
                nc.scalar.activation(sigl[:], g2a[:], AF.Sigmoid,
                                     bias=lrb_sb[0:1, 0:1], scale=8.0,
                                     accum_out=lsum[:])
                fparts = spool.tile([1, 2], F32, name=f"fparts{j}", tag="fparts")
                for b in range(B):
                    r = spool.tile([1, 1], F32, name=f"zfr{j}_{b}", tag=f"zfr{b}")
                    nc.vector.tensor_reduce(r[:], g2b[0:1, b * C:(b + 1) * C],
                                            mybir.AxisListType.X, ALU.add)
                    nc.scalar.activation(fparts[:, b:b + 1], r[:], AF.Sigmoid,
                                         bias=fgb_sb[0:1, 0:1], scale=8.0 / C)
                f11 = spool.tile([1, 1], F32, name=f"f11_{j}", tag="f11")
                nc.vector.tensor_reduce(f11[:], fparts[:], mybir.AxisListType.X, ALU.add)
                nc.vector.tensor_scalar_mul(f11[:], f11[:], 0.5)

                # scalar row: 0=negs 1=negs0 2=negs08 3=cn 4=c8n 5=-rcn
                scrow = spool.tile([1, 6], F32, name=f"scrow{j}", tag="scrow")
                nc.vector.tensor_tensor(scrow[0:1, 3:4], c11[:], f11[:], ALU.mult)
                rcn = spool.tile([1, 1], F32, name=f"rcn{j}", tag="rcn")
                nc.vector.reciprocal(rcn[:], scrow[0:1, 3:4])
                nc.vector.tensor_tensor(scrow[0:1, 0:1], lsum[:], rcn[:], ALU.mult)
                nc.vector.tensor_scalar_mul(scrow[0:1, 0:1], scrow[0:1, 0:1], K1)
                negs8 = spool.tile([1, 1], F32, name=f"negs8_{j}", tag="negs8")
                nc.vector.tensor_scalar_mul(negs8[:], scrow[0:1, 0:1], 1.0 / 8.0)
                nc.vector.tensor_tensor(scrow[0:1, 1:2], scrow[0:1, 0:1], c11[:], ALU.mult)
                nc.vector.tensor_scalar_mul(scrow[0:1, 2:3], scrow[0:1, 1:2], 8.0)
                nc.vector.tensor_scalar_mul(scrow[0:1, 4:5], scrow[0:1, 3:4], 8.0)
                nc.vector.tensor_scalar_mul(scrow[0:1, 5:6], rcn[:], -1.0)
                # broadcast all six down the partitions with one PE matmul
                psbc = psD.tile([128, 6], F32, name=f"psbc{j}", tag="psD")
                nc.tensor.matmul(psbc[:], ones_1r[0:1, :], scrow[0:1, :],
                                 start=True, stop=True)
                bcs = spool.tile([128, 6], F32, name=f"bcs{j}", tag="bcs")
                nc.vector.tensor_copy(bcs[:], psbc[:])
                identm = tp.tile([128, 128], BF, name=f"idm{j}", tag="identm")
                nc.scalar.activation(identm[:], ident[:], AF.Copy,
                                     scale=bcs[:, 5:6])
                return dict(cn11=scrow[0:1, 3:4], negs8=negs8, bcs=bcs,
                            identm=identm)

            def fwd1(xT, sc, cqb, want_hp, j, pfx, pspool=None, pstag="psB"):
                """Layer-1 m-major: hT [m,(k)*T], optional hpT. ACT per-partition
                bias; silu & silu' derived from sigmoid."""
                if pspool is None:
                    pspool = psB
                hT = hpool.tile([128, KM * T], BF, name=f"hT{pfx}_{j}", tag=f"hT{pfx}")
                hpT = None
                if want_hp:
                    hpT = hpool.tile([128, KM * T], BF, name=f"hpT_{j}", tag="hpT")
                for k in range(KM):
                    pt = pspool.tile([128, T], F32, name=f"ps1{pfx}_{j}_{k}", tag=pstag)
                    for i in range(KD):
                        nc.tensor.matmul(pt[:],
                                         q0t[:, i * MS + k * 128:i * MS + (k + 1) * 128],
                                         xT[:, i * T:(i + 1) * T],
                                         start=(i == 0), stop=(i == KD - 1))
                    sl = slice(k * T, (k + 1) * T)
                    sT = tp.tile([128, T], BF, name=f"sT{pfx}_{j}_{k}", tag=f"sT{k}")
                    nc.scalar.activation(sT[:], pt[:], AF.Sigmoid,
                                         bias=cqb[:, k:k + 1], scale=sc["bcs"][:, 4:5])
                    pre = tp.tile([128, T], BF, name=f"pre{pfx}_{j}_{k}", tag=f"pre{k}")
                    nc.scalar.activation(pre[:], pt[:], AF.Identity,
                                         bias=cqb[:, k:k + 1], scale=sc["bcs"][:, 4:5])
                    nc.vector.tensor_tensor(hT[:, sl], pre[:], sT[:], ALU.mult)
                    if want_hp:
                        m_ = tp.tile([128, T], BF, name=f"m{pfx}_{j}_{k}", tag=f"mm{k}")
                        nc.vector.tensor_tensor(m_[:], hT[:, sl], sT[:], ALU.mult)
                        v_ = tp.tile([128, T], BF, name=f"v{pfx}_{j}_{k}", tag=f"vv{k}")
                        nc.vector.tensor_tensor(v_[:], sT[:], m_[:], ALU.subtract)
                        nc.vector.tensor_tensor(hpT[:, sl], v_[:], hT[:, sl], ALU.add)
                return hT, hpT

            def fwd2_pred(hT, sc, xb_next, j):
                """Layer-2 tokens-major for pred: evac (cn*psum - x/8) -> arin DMA,
                issue AllReduce. Returns arout handle."""
                psb = tp.tile([128, KT * D], ARDT, name=f"parin{j}", tag="parin")
                arin = dr.tile([T, D], ARDT, name=f"arin{j}", tag="arin")
                for t in range(KT):
                    for n in range(NN):
                        pt = psA.tile([128, 512], F32, name=f"psp{j}_{t}_{n}", tag="psA")
                        for k in range(KM):
                            nc.tensor.matmul(pt[:], hT[:, k * T + t * 128:k * T + (t + 1) * 128],
                                             q1t[:, k * D + n * 512:k * D + (n + 1) * 512],
                                             start=(k == 0), stop=False)
                        nc.tensor.matmul(pt[:], ones_row[:], bk1[:, n * 512:(n + 1) * 512],
                                         start=False, stop=False)
                        sl = slice(t * D + n * 512, t * D + (n + 1) * 512)
                        nc.tensor.matmul(pt[:], sc["identm"][:], xb_next[:, sl],
                                         start=False, stop=True)
                        if n % 2 == 0:
                            nc.vector.tensor_scalar_mul(psb[:, sl], pt[:],
                                                        sc["bcs"][:, 3:4])
                        else:
                            nc.scalar.mul(psb[:, sl], pt[:], sc["bcs"][:, 3:4])
                    eng = nc.gpsimd if t == 0 else nc.sync
                    eng.dma_start(arin[t * 128:(t + 1) * 128, :],
                                  psb[:, t * D:(t + 1) * D])
                if no_ar:
                    return arin
                arout = dr.tile([T, D], ARDT, name=f"arout{j}", tag="arout",
                                addr_space="Shared")
                nc.gpsimd.collective_compute(
                    "AllReduce", ALU.add, replica_groups=[list(range(NCORES))],
                    ins=[arin.opt()], outs=[arout.opt()])
                return arout

            def fwd2_out(hT, sc, j):
                """Layer-2 tokens-major for out_j: evac cn*psum -> bf16 -> outp."""
                osb = tp.tile([128, KT * D], BF, name=f"osb{j}", tag="osb")
                for t in range(KT):
                    for n in range(NN):
                        pt = psO.tile([128, 512], F32, name=f"pso{j}_{t}_{n}", tag="psO")
                        for k in range(KM):
                            nc.tensor.matmul(pt[:], hT[:, k * T + t * 128:k * T + (t + 1) * 128],
                                             q1t[:, k * D + n * 512:k * D + (n + 1) * 512],
                                             start=(k == 0), stop=False)
                        nc.tensor.matmul(pt[:], ones_row[:], bk1[:, n * 512:(n + 1) * 512],
                                         start=False, stop=True)
                        sl = slice(t * D + n * 512, t * D + (n + 1) * 512)
                        nc.scalar.activation(osb[:, sl], pt[:], AF.Copy,
                                             scale=sc["bcs"][:, 3:4])
                    nc.sync.dma_start(outp[t, j * C:(j + 1) * C, :],
                                      osb[:, t * D:(t + 1) * D])

            def transpose_h(hT, j):
                """hT [m,(k)T] -> h tokens-major [t, KT*MS] (one bank, one evac)."""
                h1 = hpool.tile([128, KT * MS], BF, name=f"h1_{j}", tag="h1")
                pt = psT.tile([128, 512], BF, name=f"pth{j}", tag="psT")
                for q in range(4):      # (t,k): dst col t*MS+k*128 contiguous
                    t, k = q // 2, q % 2
                    nc.tensor.transpose(pt[:, t * MS + k * 128:t * MS + k * 128 + 128],
                                        hT[:, k * T + t * 128:k * T + (t + 1) * 128],
                                        ident[:])
                nc.vector.tensor_copy(h1[:], pt[:])
                return h1

            # ---------------- prologue ----------------
            c11 = spool.tile([1, 1], F32, name="c11", tag="c11")
            nc.vector.memset(c11[:], 1.0)
            xb_c = load_x(0)
            xT_c = transpose_x(xb_c, 0)
            sc_c = gates_and_scalars(xT_c, 0, c11)      # scalars for iteration 0
            c11 = sc_c["cn11"]
            # chunk 0 forward under P_0 (c=1): scale 8, bias qb0c
            bcs0 = spool.tile([128, 6], F32, name="bcs0")
            nc.vector.memset(bcs0[:], 0.0)
            nc.vector.memset(bcs0[:, 3:4], 1.0)
            nc.vector.memset(bcs0[:, 4:5], 8.0)
            nc.vector.memset(bcs0[:, 5:6], -1.0)
            identm0 = tp.tile([128, 128], BF, name="idm_p", tag="identm")
            nc.scalar.activation(identm0[:], ident[:], AF.Copy, scale=-1.0)
            sc0 = dict(sc_c)
            sc0["bcs"] = bcs0
            sc0["identm"] = identm0

            xb_n = load_x(1)
            xT_n = transpose_x(xb_n, 1)
            sc_n = gates_and_scalars(xT_n, 1, c11)      # scalars for iteration 1
            c11 = sc_n["cn11"]
            cqb = spool.tile([128, KM], F32, name="cqb_p", tag="cqb")
            nc.vector.tensor_copy(cqb[:], qb0c[:])
            hT_c, hpT_c = fwd1(xT_c, sc0, cqb, True, 0, "1")
            h1_c = transpose_h(hT_c, 0)
            ar_cur = fwd2_pred(hT_c, sc0, xb_c, 0)

            xb_f = load_x(2)
            xT_f = transpose_x(xb_f, 2)

            # ---------------- main loop ----------------
            for j in range(NCH):
                last = (j == NCH - 1)
                sc = sc_c   # scalars for iteration j (gates of chunk j)

                # AllReduce result -> SBUF (dpred, tokens-major)
                dpred = tp.tile([128, KT * D], BF, name=f"dp{j}", tag="dp")
                for t in range(KT):
                    if ar_fp8:
                        eng = nc.gpsimd
                    else:
                        eng = nc.sync if t == 0 else nc.scalar
                    eng.dma_start(dpred[:, t * D:(t + 1) * D],
                                  ar_cur[t * 128:(t + 1) * 128, :])

                # dpredT (d-major) via grouped PE transposes
                dpT = tp.tile([128, KD * T], BF, name=f"dpT{j}", tag="dpT")
                for i2 in range(KD // 2):
                    pt = psT.tile([128, 512], BF, name=f"ptd{j}_{i2}", tag="psT")
                    for q in range(4):
                        i = 2 * i2 + q // 2
                        t = q % 2
                        nc.tensor.transpose(pt[:, q * 128:(q + 1) * 128],
                                            dpred[:, t * D + i * 128:t * D + (i + 1) * 128],
                                            ident[:])
                    with tc.high_priority():
                        if i2 % 2 == 0:
                            nc.vector.tensor_copy(dpT[:, i2 * 512:(i2 + 1) * 512], pt[:])
                        else:
                            nc.scalar.copy(dpT[:, i2 * 512:(i2 + 1) * 512], pt[:])

                # dhT (m-major) = q1n^T-contraction; dhpT = dhT * hpT
                dhpT = tp.tile([128, KM * T], BF, name=f"dhpT{j}", tag="dhpT")
                for k in range(KM):
                    pt = psB.tile([128, T], F32, name=f"psdh{j}_{k}", tag="psB")
                    for i in range(KD):
                        nc.tensor.matmul(pt[:],
                                         q1n[:, i * MS + k * 128:i * MS + (k + 1) * 128],
                                         dpT[:, i * T:(i + 1) * T],
                                         start=(i == 0), stop=(i == KD - 1))
                    sl = slice(k * T, (k + 1) * T)
                    with tc.high_priority():
                        nc.vector.tensor_tensor(dhpT[:, sl], pt[:], hpT_c[:, sl], ALU.mult)

                # gb0 column via DVE free-axis reduce of dhpT; update qb0c; cqb0
                gb0c = spool.tile([128, KM], F32, name=f"gb0c{j}", tag="gb0c")
                cqb = spool.tile([128, KM], F32, name=f"cqb{j}", tag="cqb")
                with tc.high_priority():
                    for k in range(KM):
                        nc.vector.tensor_reduce(gb0c[:, k:k + 1], dhpT[:, k * T:(k + 1) * T],
                                                mybir.AxisListType.X, ALU.add)
                    nc.vector.scalar_tensor_tensor(qb0c[:], gb0c[:], sc["bcs"][:, 1:2],
                                                   qb0c[:], ALU.mult, ALU.add)
                    nc.vector.scalar_tensor_tensor(cqb[:], qb0c[:], sc["bcs"][:, 3:4],
                                                   ones_t[:, 0:KM], ALU.mult, ALU.mult)

                # dhp tokens-major (one bank, one evac)
                dhp = tp.tile([128, KT * MS], BF, name=f"dhp{j}", tag="dhp")
                ptd = psT.tile([128, 512], BF, name=f"ptdh{j}", tag="psT")
                for q in range(4):
                    t, k = q // 2, q % 2
                    nc.tensor.transpose(ptd[:, t * MS + k * 128:t * MS + k * 128 + 128],
                                        dhpT[:, k * T + t * 128:k * T + (t + 1) * 128],
                                        ident[:])
                with tc.high_priority():
                    nc.vector.tensor_copy(dhp[:], ptd[:])

                # gW0 (q0t layout) fused updates (x/8 lhsT -> scale *8)
                for i in range(KD):
                    pt = psB.tile([128, MS], F32, name=f"psg0_{j}_{i}", tag="psB")
                    for t in range(KT):
                        nc.tensor.matmul(pt[:], xb_c[:, t * D + i * 128:t * D + (i + 1) * 128],
                                         dhp[:, t * MS:(t + 1) * MS],
                                         start=(t == 0), stop=(t == KT - 1))
                    sl = slice(i * MS, (i + 1) * MS)
                    if i % 2 == 0:
                        nc.vector.scalar_tensor_tensor(q0t[:, sl], pt[:],
                                                       sc["bcs"][:, 2:3],
                                                       q0t[:, sl], ALU.mult, ALU.add)
                    else:
                        gsc = tp.tile([128, MS], BF, name=f"g0s{j}_{i}", tag="g0s")
                        nc.scalar.mul(gsc[:], pt[:], sc["bcs"][:, 2:3])
                        nc.vector.tensor_tensor(q0t[:, sl], gsc[:], q0t[:, sl],
                                                ALU.add)

                # ---- forward chunk j+1 under P_{j+1} (layer 1) ----
                if not last:
                    hT_n, hpT_n = fwd1(xT_n, sc, cqb, True, j + 1, "1")

                # gW1t (q1t layout) + gb1 from dpred/h1 -> fused updates
                # (only mm2 needs these; scheduled after fwd1 so the DVE queue
                # drains the layer-1 critical chain first)
                for k in range(KM):
                    for n in range(NN):
                        pt = psA.tile([128, 512], F32, name=f"psg1_{j}_{k}_{n}", tag="psA")
                        for t in range(KT):
                            nc.tensor.matmul(pt[:],
                                             h1_c[:, t * MS + k * 128:t * MS + (k + 1) * 128],
                                             dpred[:, t * D + n * 512:t * D + (n + 1) * 512],
                                             start=(t == 0), stop=(t == KT - 1))
                        sl = slice(k * D + n * 512, k * D + (n + 1) * 512)
                        if n % 2 == 0:
                            nc.vector.scalar_tensor_tensor(q1t[:, sl], pt[:],
                                                           sc["bcs"][:, 0:1],
                                                           q1t[:, sl], ALU.mult, ALU.add)
                        else:
                            g1s = tp.tile([128, 512], BF, name=f"g1s{j}_{k}_{n}", tag="g1s")
                            nc.scalar.mul(g1s[:], pt[:], sc["bcs"][:, 0:1])
                            nc.vector.tensor_tensor(q1t[:, sl], g1s[:], q1t[:, sl],
                                                    ALU.add)
                for n in range(NN):
                    gb1p = psD.tile([1, 512], F32, name=f"gb1_{j}_{n}", tag="psD")
                    for t in range(KT):
                        nc.tensor.matmul(gb1p[:], ones_col[:],
                                         dpred[:, t * D + n * 512:t * D + (n + 1) * 512],
                                         start=(t == 0), stop=(t == KT - 1))
                    nc.vector.scalar_tensor_tensor(bk1[0:1, n * 512:(n + 1) * 512],
                                                   gb1p[:], sc["negs8"][0:1, 0:1],
                                                   bk1[0:1, n * 512:(n + 1) * 512],
                                                   ALU.mult, ALU.add)

                # ---- layer 2 for pred_{j+1}; issue AR ----
                if not last:
                    ar_cur = fwd2_pred(hT_n, sc, xb_n, j + 1)

                # ---- shadow work (during AR flight) ----
                # re-forward chunk j -> out_j
                hT_o, _ = fwd1(xT_c, sc, cqb, False, j, "2", pspool=psO, pstag="psO")
                fwd2_out(hT_o, sc, j)

                # refresh q1n by transposing the updated q1t (needed only by
                # the next backward; exact same bf16 values as a dual update)
                for i2 in range(KD // 2):
                    pt = psT.tile([128, 512], BF, name=f"ptq{j}_{i2}", tag="psT")
                    for q in range(4):
                        i = 2 * i2 + q // 2
                        k = q % 2
                        nc.tensor.transpose(pt[:, q * 128:(q + 1) * 128],
                                            q1t[:, k * D + i * 128:k * D + (i + 1) * 128],
                                            ident[:])
                    if i2 % 2 == 0:
                        nc.vector.tensor_copy(q1n[:, i2 * 512:(i2 + 1) * 512], pt[:])
                    else:
                        nc.scalar.copy(q1n[:, i2 * 512:(i2 + 1) * 512], pt[:])

                if not last:
                    # h1 for next backward; prefetch + gates two chunks ahead
                    h1_n = transpose_h(hT_n, j + 1)
                    if j < NCH - 2:
                        sc_f = gates_and_scalars(xT_f, j + 2, c11)
                        c11 = sc_f["cn11"]
                        if j < NCH - 3:
                            xb_2 = load_x(j + 3)
                            xT_2 = transpose_x(xb_2, j + 3)
                        else:
                            xb_2 = xT_2 = None
                    else:
                        sc_f = xb_2 = xT_2 = None
                    xb_c, xT_c = xb_n, xT_n
                    xb_n, xT_n = xb_f, xT_f
                    xb_f, xT_f = xb_2, xT_2
                    hT_c, hpT_c, h1_c = hT_n, hpT_n, h1_n
                    sc_c, sc_n = sc_n, sc_f
    nc.compile()
    return nc


_NC_CACHE = None


def _get_nc():
    global _NC_CACHE
    if _NC_CACHE is None:
        _NC_CACHE = build()
    return _NC_CACHE


def make_in_maps(x, W0, b0, W1, b1, lr_w, lr_b, fg_w, fg_b):
    import ml_dtypes
    x8 = np.ascontiguousarray(
        (np.asarray(x, np.float32) * 0.125).astype(ml_dtypes.bfloat16))
    W0 = np.asarray(W0, np.float32)
    W1 = np.asarray(W1, np.float32)
    lrfg = np.ascontiguousarray(
        np.stack([np.asarray(lr_w, np.float32)[0], np.asarray(fg_w, np.float32)[0]], axis=1))
    ident = np.eye(128, dtype=np.float32)
    in_maps = []
    for s in range(NCORES):
        sl = slice(s * MS, (s + 1) * MS)
        in_maps.append({
            "x8": x8,
            "w0t": np.ascontiguousarray(W0[sl, :].T),
            "w1t": np.ascontiguousarray(W1[:, sl].T),
            "w1n": np.ascontiguousarray(W1[:, sl]),
            "lrfg": lrfg,
            "lrb": np.asarray(lr_b, np.float32).reshape(1, 1),
            "fgb": np.asarray(fg_b, np.float32).reshape(1, 1),
            "b0c": np.ascontiguousarray(np.asarray(b0, np.float32)[sl].reshape(MS, 1)),
            "b1d8": np.ascontiguousarray((np.asarray(b1, np.float32) / 8.0).reshape(1, D)),
            "ident": ident,
        })
    return in_maps


def run(inputs, **kw):
    nc = _get_nc()
    in_maps = make_in_maps(**inputs)
    res = run_bass_kernel_spmd(nc, in_maps, core_ids=list(range(NCORES)), **kw)
    out = np.zeros((B, L, D), np.float32)
    for r in res.results:
        out += r["outp"].astype(np.float32)
    return out, res


def kernel(**inputs) -> np.ndarray:
    out, _ = run(inputs)
    return out
